# revision 9
# baseline (speedup 1.0000x reference)
"""Trainium2 Bass kernel for nn_Loss_19980187861563.

Loss = NLL + coverage + gamma2 + IPOT-OT over pred = softmax(output_mle) @ W_emb.

Key algebraic facts used (verified against the reference to float32 identity):
  * The IPOT recursion `Tm = dvec * Q * sigma.T * eye` makes Tm diagonal after
    iteration 1, and the fixed point gives diag(Tm) == 1/n for every iteration
    >= 2 (max_iter=400 >> 2).  Hence ot = sum(Tm*C) = trace(C)/n, i.e. the mean
    cosine similarity between pred rows and target-embedding rows.
  * Cosine similarity is invariant to positive row scaling, so the softmax
    normalizer (and max-subtraction) cancels: only P = exp(logits) @ W_emb is
    needed, accumulated in fp32.

Device work (8 NeuronCores, vocab-sharded ~6283 columns each, padded to 6400):
  per core: P_k[512,512] = exp(x_slice[512,6400]) @ W_slice[6400,512]
            (bf16 operands, fp32 PSUM accumulation; exp on ACT, transpose of
            E-chunks via TensorE identity-matmul, 128-deep matmul accumulation)
            plus the coverage partial: column-sums of min(attn, coverage) over
            this core's 256 (b,lsrc) rows.
Host work: slice/pad/cast inputs, sum the 8 fp32 partials, cosine + NLL +
           masking + final scalar combine (microseconds of numpy).
"""

import sys

for _p in ("/opt/trn_rl_repo",):
    if _p not in sys.path:
        sys.path.insert(0, _p)

import numpy as np
import ml_dtypes

import concourse.bass as bass
import concourse.tile as tile
from concourse import bacc, mybir
from concourse.bass import ts
from concourse.bass_utils import run_bass_kernel_spmd
from concourse.masks import make_identity

BF16 = ml_dtypes.bfloat16

B, T, V, LSRC, D = 4, 128, 50257, 512, 512
NTOK = B * T                 # 512 token rows
NCORE = 8
VPC = 6283                   # vocab columns per core (last core: 6276)
VS = 6400                    # padded per-core vocab width = 50 chunks of 128
NCH = VS // 128              # 50 contraction chunks
GS = 10                      # chunks per exp/DMA group
NGRP = NCH // GS             # 5 groups
GW = GS * 128                # 1280 columns per group
PAD_ID = 0
GAMMA1, GAMMA2 = 1.0, 0.1

_BUILT = None
LAST_RESULTS = None          # BassKernelResults of the most recent run (for test.py)


def _build():
    global _BUILT
    if _BUILT is not None:
        return _BUILT

    f32 = mybir.dt.float32
    bf16 = mybir.dt.bfloat16

    # Bacc (not raw Bass): its compile() runs generate_event_semaphores,
    # which splits sync waits to the 1-wait-per-instruction HW constraint.
    nc = bacc.Bacc("TRN2", target_bir_lowering=False, debug=False,
                   num_devices=NCORE)
    x = nc.dram_tensor("x", [NTOK, VS], bf16, kind="ExternalInput").ap()
    w = nc.dram_tensor("w", [VS, D], bf16, kind="ExternalInput").ap()
    ac = nc.dram_tensor("ac", [512, T], f32, kind="ExternalInput").ap()
    p = nc.dram_tensor("p", [NTOK, D], f32, kind="ExternalOutput").ap()
    cov = nc.dram_tensor("cov", [1, T], f32, kind="ExternalOutput").ap()

    with tile.TileContext(nc) as tc:
        with (
            tc.tile_pool(name="const", bufs=1) as cpool,
            # DMA destinations get single-use slots: HWDGE DMA instructions
            # only support one sync-wait, and slot reuse would need WAR+WAW
            tc.tile_pool(name="xin", bufs=4 * NGRP) as xpool,
            tc.tile_pool(name="exp", bufs=2) as epool,
            tc.tile_pool(name="win", bufs=NCH) as wpool,
            tc.tile_pool(name="et", bufs=3) as etpool,
            tc.tile_pool(name="covs", bufs=2) as covpool,
            tc.tile_pool(name="tp", bufs=2, space="PSUM") as tppool,
            tc.tile_pool(name="acc", bufs=1, space="PSUM") as apool,
            tc.tile_pool(name="covp", bufs=1, space="PSUM") as cppool,
        ):
            ident = cpool.tile([128, 128], bf16, tag="ident")
            make_identity(nc, ident[:])
            ones = cpool.tile([128, 1], f32, tag="ones")
            nc.gpsimd.memset(ones[:], 1.0)

            acc = [apool.tile([128, D], f32, tag=f"acc{t}", name=f"acc{t}")
                   for t in range(4)]

            for g in range(NGRP):
                # stage this group's logits + exp (bf16 in, bf16 out)
                e_cur = []
                for t in range(4):
                    xt = xpool.tile([128, GW], bf16, tag="xt")
                    nc.sync.dma_start(xt[:], x[ts(t, 128), ts(g, GW)])
                    et = epool.tile([128, GW], bf16, tag=f"e{t}")
                    nc.scalar.activation(et[:], xt[:],
                                         mybir.ActivationFunctionType.Exp)
                    e_cur.append(et)

                for ci in range(GS):
                    c = g * GS + ci
                    wt = wpool.tile([128, D], bf16, tag="wt")
                    nc.sync.dma_start(wt[:], w[ts(c, 128), :])

                    # transpose the four [tok=128, v=128] E chunks into one
                    # [v=128, tok=512] PSUM tile, then one DVE copy to SBUF
                    tp = tppool.tile([128, NTOK], bf16, tag="tp")
                    for t in range(4):
                        nc.tensor.transpose(tp[:, ts(t, 128)],
                                            e_cur[t][:, ts(ci, 128)],
                                            ident[:])
                    ett = etpool.tile([128, NTOK], bf16, tag="ett")
                    nc.vector.tensor_copy(ett[:], tp[:])

                    for t in range(4):
                        nc.tensor.matmul(acc[t][:], ett[:, ts(t, 128)], wt[:],
                                         start=(c == 0), stop=(c == NCH - 1))

            for t in range(4):
                po = etpool.tile([128, D], f32, tag="pout", bufs=2)
                nc.vector.tensor_copy(po[:], acc[t][:])
                nc.sync.dma_start(p[ts(t, 128), :], po[:])

            # coverage partial: rows 0-255 of ac = attn, 256-511 = coverage
            covp = cppool.tile([1, T], f32, tag="covp")
            for i in range(2):
                at = covpool.tile([128, T], f32, tag="at")
                nc.sync.dma_start(at[:], ac[ts(i, 128), :])
                ct = covpool.tile([128, T], f32, tag="ct")
                nc.sync.dma_start(ct[:], ac[ts(i + 2, 128), :])
                mt = covpool.tile([128, T], f32, tag="mt")
                nc.vector.tensor_tensor(mt[:], at[:], ct[:],
                                        op=mybir.AluOpType.min)
                nc.tensor.matmul(covp[:], ones[:], mt[:],
                                 start=(i == 0), stop=(i == 1))
            co = covpool.tile([1, T], f32, tag="covout", bufs=1)
            nc.vector.tensor_copy(co[:], covp[:])
            nc.sync.dma_start(cov[:], co[:])

    nc.compile()
    _BUILT = nc
    return nc


def kernel(output_mle, attn_dist, coverage, trg, dec_mask, dec_len, W_emb):
    global LAST_RESULTS
    om = np.ascontiguousarray(np.asarray(output_mle, dtype=np.float32))
    ad = np.asarray(attn_dist, dtype=np.float32)
    cv = np.asarray(coverage, dtype=np.float32)
    trg = np.asarray(trg)
    dm = np.asarray(dec_mask)
    dl = np.asarray(dec_len)
    W = np.ascontiguousarray(np.asarray(W_emb, dtype=np.float32))

    flat = om.reshape(NTOK, V)
    xbf = flat.astype(BF16)
    wbf = W.astype(BF16)
    ad2 = ad.reshape(B * LSRC, T)
    cv2 = cv.reshape(B * LSRC, T)

    in_maps = []
    for k in range(NCORE):
        v0 = k * VPC
        v1 = min(v0 + VPC, V)
        n = v1 - v0
        xk = np.zeros((NTOK, VS), dtype=BF16)
        xk[:, :n] = xbf[:, v0:v1]
        wk = np.zeros((VS, D), dtype=BF16)
        wk[:n] = wbf[v0:v1]
        ack = np.concatenate([ad2[k * 256:(k + 1) * 256],
                              cv2[k * 256:(k + 1) * 256]], axis=0)
        in_maps.append({"x": xk, "w": wk,
                        "ac": np.ascontiguousarray(ack, dtype=np.float32)})

    res = run_bass_kernel_spmd(_build(), in_maps, core_ids=list(range(NCORE)))
    LAST_RESULTS = res

    P = np.zeros((NTOK, D), dtype=np.float32)
    covp = np.zeros((B, T), dtype=np.float32)
    for k in range(NCORE):
        P += res.results[k]["p"]
        covp[k // 2] += res.results[k]["cov"][0]

    # --- NLL ---
    trgf = trg.reshape(-1).astype(np.int64)
    tok_lp = np.log(flat[np.arange(NTOK), trgf])
    valid = trgf != PAD_ID
    nll = -tok_lp[valid].sum(dtype=np.float32) / np.float32(valid.sum())

    # --- coverage ---
    covm = np.where(dm.reshape(B, T), np.float32(0), covp)
    cov_loss = covm.sum(dtype=np.float32) / np.float32(dl.sum())

    # --- OT = mean cosine(pred_i, trg_emb_i); row scaling cancels ---
    temb = W[trgf]
    Pn = P / np.linalg.norm(P, axis=1, keepdims=True)
    Tn = temb / np.linalg.norm(temb, axis=1, keepdims=True)
    ot = (Pn * Tn).sum(axis=1).sum(dtype=np.float32) / np.float32(NTOK)

    total = np.float32(nll + np.float32(GAMMA1) * cov_loss
                       + np.float32(GAMMA2) + ot)
    return np.asarray(total, dtype=np.float32)
